# revision 19
# baseline (speedup 1.0000x reference)
"""Trainium2 Bass kernel for a 4-layer GCN (PyG GCNConv semantics).

Math: each layer computes  h' = relu(A_hat @ h @ W + b)  where
A_hat = D^-1/2 A D^-1/2 + D^-1 (self loops), D = in-degree + 1.
Aggregation commutes with the dense transform, so each layer aggregates in
whichever of (in_dim, out_dim) is cheaper:
  L1: aggregate x (width 2, host-permuted stream), then @W1      -> h1 [N,128]
  L2: gather h1 rows (256B bf16), segment-sum, @W2               -> h2 [N,256]
  L3: gather h2 rows (512B bf16), segment-sum, @W3, fuse t=h3@W4 -> t  [N,2]
  L4: gather t rows (256B bf16 padded), segment-sum, + b4        -> out [N,2]

Normalization is separable: w_uv = dinv[u]*dinv[v].  Tables store
dinv[u]*h[u] (src factor folded in at the producing epilogue); the dst
factor dinv[v] is applied post-GEMM per block (diag-left commutes with @W).
Self-loops are then plain edges.  One-hot scatter matrices are pure binary
(iota == dstslot), built batched on DVE; pad slots use dstslot=-1.

Sharding: destination-node slabs. Core c owns 49 blocks x 128 dst nodes.
Edges (+self loops) are grouped per dst block, split lo/hi at src<32768
(dma_gather idx is int16), sorted by src, padded to 128-edge tiles.

Gathers run on all 4 SWDGE queues round-robin (4 Q7 pairs generate
descriptors concurrently; ~2.9x descgen throughput vs one queue).
Tables are bf16 (halves gather bytes); PSUM accumulation stays fp32.
Tile counts per (block, phase) are max'd across the 8 cores so one NEFF
serves all cores (SPMD).
"""

import sys

for _p in ("/opt/trn_rl_repo",):
    if _p not in sys.path:
        sys.path.insert(0, _p)

from contextlib import ExitStack

import numpy as np

import concourse.bass as bass
import concourse.bacc as bacc
import concourse.mybir as mybir
import concourse.tile as tile
from concourse import library_config
from concourse.masks import make_identity

P = 128
NCORES = 8
F32 = mybir.dt.float32
BF16 = mybir.dt.bfloat16
I16 = mybir.dt.int16
I32 = mybir.dt.int32
NQ = 4  # SWDGE queues


class GCNConfig:
    def __init__(self, n_nodes, dims, blocks_per_core, bpc_a):
        self.n_nodes = n_nodes
        self.dims = list(dims)  # [2, 128, 256, 512, 2]
        self.bpc = blocks_per_core
        self.bpc_a = bpc_a          # blocks in slab half A (rest in half B)
        self.slab = blocks_per_core * P
        self.npad = NCORES * self.slab
        self.ha = bpc_a * P         # rows per core in half A
        self.hb = self.slab - self.ha
        self.na = NCORES * self.ha  # rows in table A (= concat of A halves)
        self.nb = NCORES * self.hb
        assert self.npad >= n_nodes
        assert self.na < 32768 and self.nb < 32768  # int16 gather indices
        self.tpad = 128  # bf16 cols in the padded width-2 "t" table (256B row)


REAL_CFG = GCNConfig(n_nodes=50000, dims=[2, 128, 256, 512, 2],
                     blocks_per_core=49, bpc_a=24)


# --------------------------------------------------------------------------
# Host-side graph preprocessing
# --------------------------------------------------------------------------

def preprocess(cfg, edge_index, x):
    """Shard + tile the graph. Returns (tiles [bpc,2] int, per-core dict of
    host arrays)."""
    src = np.asarray(edge_index[0], dtype=np.int64)
    dst = np.asarray(edge_index[1], dtype=np.int64)
    n = cfg.n_nodes
    deg = np.bincount(dst, minlength=n).astype(np.float32) + 1.0
    dinv = 1.0 / np.sqrt(deg)
    dinv_pad = np.ones(cfg.npad, np.float32)
    dinv_pad[:n] = dinv

    es = np.concatenate([src, np.arange(n, dtype=np.int64)])
    ed = np.concatenate([dst, np.arange(n, dtype=np.int64)])

    blk = ed // P
    owner = es // cfg.slab
    r_in_slab = es % cfg.slab
    hi = (r_in_slab >= cfg.ha).astype(np.int64)
    # row index within half-table A or B (tables are concat of slab halves)
    erow = np.where(hi == 1, owner * cfg.hb + (r_in_slab - cfg.ha),
                    owner * cfg.ha + r_in_slab)
    order = np.lexsort((es, hi, blk))
    es, ed, blk, hi, erow = (es[order], ed[order], blk[order], hi[order],
                             erow[order])

    nblocks = NCORES * cfg.bpc
    cnt = np.zeros((nblocks, 2), np.int64)
    np.add.at(cnt, (blk, hi), 1)
    cnt_core = cnt.reshape(NCORES, cfg.bpc, 2)
    tiles = (-(-cnt_core // P)).max(axis=0)  # [bpc, 2] ceil-div then max
    tiles = np.maximum(tiles, 1)  # both phases always present
    tt = int(tiles.sum())

    # group start offsets in the sorted edge arrays, per (block, phase)
    starts = np.zeros(nblocks * 2 + 1, np.int64)
    np.add.at(starts, blk * 2 + hi + 1, 1)
    starts = np.cumsum(starts)

    # L1 aggregation on host (width-2 segment-sum; same class of host prep
    # as the per-edge x[src] gather the stream variant needed):
    #   agg1[v] = sum_{e in E+self, dst=v} dinv[src] * x[src]
    xf = np.asarray(x, dtype=np.float32)
    xw = xf * dinv[:, None]  # [n, 2]
    agg1 = np.zeros((cfg.npad, 2), np.float32)
    np.add.at(agg1, ed, xw[es])

    per_core = []
    for c in range(NCORES):
        idx = np.zeros((tt, P), np.int16)
        dl = np.full((tt, P), -1.0, np.float32)  # -1 => zero one-hot col
        cur = 0
        for j in range(cfg.bpc):
            g = c * cfg.bpc + j
            for ph in range(2):
                t = int(tiles[j, ph])
                if t == 0:
                    continue
                s0, s1 = starts[g * 2 + ph], starts[g * 2 + ph + 1]
                ne = s1 - s0
                assert ne <= t * P
                flat_i = np.zeros(t * P, np.int64)
                flat_d = np.full(t * P, -1.0, np.float32)
                flat_i[:ne] = erow[s0:s1]
                flat_d[:ne] = (ed[s0:s1] - g * P).astype(np.float32)
                sl = slice(cur, cur + t)
                idx[sl] = flat_i.reshape(t, P).astype(np.int16)
                dl[sl] = flat_d.reshape(t, P)
                cur += t
        assert cur == tt
        # device layouts:
        # idx: [128, tt*8] int16 -- per tile, idx i at [i%16 (replic 8x), i//16]
        idx_dev = np.zeros((P, tt * 8), np.int16)
        for rep in range(8):
            idx_dev[rep * 16:(rep + 1) * 16] = (
                idx.reshape(tt, 8, 16).transpose(2, 0, 1).reshape(16, tt * 8))
        # dinv columns for this core's slab: [128, bpc]
        dslab = dinv_pad[c * cfg.slab:(c + 1) * cfg.slab].reshape(cfg.bpc, P).T
        per_core.append({
            "eidx": idx_dev,
            "edl": dl.T.copy(),                      # [128, tt]
            "eag1": np.ascontiguousarray(
                agg1[c * cfg.slab:(c + 1) * cfg.slab].T),  # [2, slab]
            "dinv": np.ascontiguousarray(dslab),     # [128, bpc]
        })
    return tiles, per_core


# --------------------------------------------------------------------------
# Device module builder
# --------------------------------------------------------------------------

def build_module(cfg, tiles, debug_layers=4):
    d0, d1, d2, d3, d4 = cfg.dims
    bpc = cfg.bpc
    tt = int(tiles.sum())
    nc = bacc.Bacc(None, target_bir_lowering=False, num_swdge_queues=NQ,
                   dynamic_dma_scratch_size=49152)

    eidx = nc.declare_dram_parameter("eidx", [P, tt * 8], I16, False)
    edl = nc.declare_dram_parameter("edl", [P, tt], F32, False)
    eag1 = nc.declare_dram_parameter("eag1", [d0, cfg.slab], F32, False)
    dinv_p = nc.declare_dram_parameter("dinv", [P, bpc], F32, False)
    W1 = nc.declare_dram_parameter("W1", [d0, d1], F32, False)
    W2 = nc.declare_dram_parameter("W2", [d1, d2], F32, False)
    W3 = nc.declare_dram_parameter("W3", [d2, d3], F32, False)
    W4 = nc.declare_dram_parameter("W4", [d3, d4], F32, False)
    b1 = nc.declare_dram_parameter("b1", [1, d1], F32, False)
    b2 = nc.declare_dram_parameter("b2", [1, d2], F32, False)
    b3 = nc.declare_dram_parameter("b3", [1, d3], F32, False)
    b4 = nc.declare_dram_parameter("b4", [1, d4], F32, False)
    out_p = nc.declare_dram_parameter("out", [cfg.slab, d4], F32, True)

    rg = [list(range(NCORES))]
    eq = mybir.AluOpType.is_equal
    mul = mybir.AluOpType.mult
    add = mybir.AluOpType.add
    relu = mybir.ActivationFunctionType.Relu

    qctr = [0]

    def next_q():
        q = qctr[0] % NQ
        qctr[0] += 1
        return q

    with tile.TileContext(nc, num_cores=NCORES) as tc, ExitStack() as ctx:
        dram = ctx.enter_context(tc.tile_pool(name="dram", bufs=1, space="DRAM"))
        t1a = dram.tile([cfg.na, d1], BF16, addr_space="Shared")
        t1b = dram.tile([cfg.nb, d1], BF16, addr_space="Shared")
        t2a = dram.tile([cfg.na, d2], BF16, addr_space="Shared")
        t2b = dram.tile([cfg.nb, d2], BF16, addr_space="Shared")
        tta = dram.tile([cfg.na, cfg.tpad], BF16, addr_space="Shared")
        ttb = dram.tile([cfg.nb, cfg.tpad], BF16, addr_space="Shared")
        ag1 = dram.tile([cfg.slab, d1], BF16)
        ag2 = dram.tile([cfg.slab, d2], BF16)
        agt = dram.tile([cfg.slab, cfg.tpad], BF16)

        const = ctx.enter_context(tc.tile_pool(name="const", bufs=1))
        iota_i = const.tile([P, P], I32)
        iota_f = const.tile([P, P], F32)
        nc.gpsimd.iota(iota_i[:], pattern=[[1, P]], base=0, channel_multiplier=0)
        nc.vector.tensor_copy(iota_f[:], iota_i[:])
        iota_rep = const.tile([P, 8 * P], F32)
        for k in range(8):
            nc.vector.tensor_copy(iota_rep[:, k * P:(k + 1) * P], iota_f[:])
        ident = const.tile([P, P], BF16)
        make_identity(nc, ident[:])
        id2 = const.tile([d4, d4], BF16)
        make_identity(nc, id2[:])
        ones = const.tile([1, P], BF16)
        nc.vector.memset(ones[:], 1.0)
        # iota (standard lib) is done; switch GPSIMD ucode to the library
        # that provides DMAGatherAnt
        nc.gpsimd.load_library(library_config.mlp)

        # weights as bf16 in SBUF
        w1s = const.tile([d0, d1], BF16)
        nc.gpsimd.dma_start(out=w1s[:], in_=W1[:, :])
        w2s = const.tile([d1, d2], BF16)
        nc.gpsimd.dma_start(out=w2s[:], in_=W2[:, :])
        w3s = [const.tile([P, d3], BF16, tag=f"w3_{k}", name=f"w3_{k}") for k in range(d2 // P)]
        for k in range(d2 // P):
            nc.gpsimd.dma_start(out=w3s[k][:], in_=W3[k * P:(k + 1) * P, :])
        w4s = [const.tile([P, d4], BF16, tag=f"w4_{k}", name=f"w4_{k}") for k in range(d3 // P)]
        for k in range(d3 // P):
            nc.gpsimd.dma_start(out=w4s[k][:], in_=W4[k * P:(k + 1) * P, :])
        # bias rows (bf16 for the ones-matmul trick)
        brs = []
        for name, bparam, od in (("b1", b1, d1), ("b2", b2, d2),
                                 ("b3", b3, d3), ("b4", b4, d4)):
            r = const.tile([1, od], BF16, tag=name + "s", name=name + "s")
            nc.gpsimd.dma_start(out=r[:], in_=bparam[:, :])
            brs.append(r)
        b1r, b2r, b3r, b4r = brs
        # bias broadcast to all partitions (fp32 SBUF), one per layer
        bsbs = []
        with tc.tile_pool(name="bp", bufs=1, space="PSUM") as bpp:
            for name, br, od in (("b1", b1r, d1), ("b2", b2r, d2),
                                 ("b3", b3r, d3), ("b4", b4r, d4)):
                ps = bpp.tile([P, od], F32, tag="bps", name=f"bps_{name}")
                nc.tensor.matmul(out=ps[:], lhsT=ones[:], rhs=br[:],
                                 start=True, stop=True)
                sb = const.tile([P, od], F32, tag=name + "sb", name=name + "sb")
                nc.scalar.copy(out=sb[:], in_=ps[:])
                bsbs.append(sb)
        b1sb, b2sb, b3sb, b4sb = bsbs

        dvs = const.tile([P, bpc], F32)
        nc.sync.dma_start(out=dvs[:], in_=dinv_p[:, :])
        esb = const.tile([P, tt * 8], I16)
        nc.sync.dma_start(out=esb[:], in_=eidx[:, :])
        dls = const.tile([P, tt], F32)
        nc.sync.dma_start(out=dls[:], in_=edl[:, :])
        ag1s = const.tile([d0, cfg.slab], BF16)
        nc.gpsimd.dma_start(out=ag1s[:], in_=eag1[:, :])

        out_acc = const.tile([P, d4 * bpc], F32)
        nc.vector.memset(out_acc[:], 0.0)

        def build_onehots(ohp, j, cur, t_all):
            """Batched binary one-hot tiles for block j: [128, t_all*128] bf16."""
            ohb = ohp.tile([P, t_all, P], BF16, tag="ohb")
            for g0 in range(0, t_all, 8):
                g = min(8, t_all - g0)
                nc.vector.tensor_tensor(
                    out=ohb[:, g0:g0 + g, :],
                    in0=iota_rep[:].rearrange("p (n q) -> p n q", q=P)[:, 0:g, :],
                    in1=dls[:, cur + g0:cur + g0 + g]
                        .rearrange("p (n u) -> p n u", u=1).to_broadcast([P, g, P]),
                    op=eq)
            return ohb

        # per-block tile offsets into the shared (eidx, dls) streams
        boffs = []
        _cur = 0
        for j in range(bpc):
            _tlo, _thi = int(tiles[j, 0]), int(tiles[j, 1])
            boffs.append((_cur, _tlo, _thi))
            _cur += _tlo + _thi

        def seg_blocks(lname, taba, tabb, elem, icols, epilogue,
                       post_block=None, msg_bufs=4):
            """Two passes over dst blocks: pass A gathers from the half-A
            table, accumulating per block into an SBUF buffer; pass B gathers
            from half-B, adds, and runs the epilogue.  Pass A only depends on
            AG_A of the previous layer, so AG_B overlaps pass A."""
            chunks = [(k * P, min(icols, (k + 1) * P))
                      for k in range(-(-icols // P))]
            with tc.tile_pool(name=lname + "m", bufs=msg_bufs) as mp, \
                 tc.tile_pool(name=lname + "oh", bufs=3) as ohp, \
                 tc.tile_pool(name=lname + "ps", bufs=2, space="PSUM") as pp, \
                 tc.tile_pool(name=lname + "ac", bufs=1) as accp, \
                 tc.tile_pool(name=lname + "as", bufs=2) as asp:
                acc = [accp.tile([c1 - c0, bpc * P], F32, tag=f"acc{k}",
                                 name=f"acc{k}")
                       for k, (c0, c1) in enumerate(chunks)]

                def halfpass(pass_b):
                    for j in range(bpc):
                        cur0, tlo, thi = boffs[j]
                        cur = cur0 + (tlo if pass_b else 0)
                        tp = thi if pass_b else tlo
                        tab = tabb if pass_b else taba
                        msg = mp.tile([P, tp, elem], BF16,
                                      tag="msgB" if pass_b else "msgA")
                        # ring limit: with the enlarged DMA scratch carveout
                        # (48KB/partition) 8-tile calls fit all 4 queues.
                        for k0 in range(0, tp, 8):
                            kt = min(8, tp - k0)
                            nc.gpsimd.dma_gather(
                                out_ap=msg[:, k0:k0 + kt, :],
                                in_ap=tab,
                                idxs_ap=esb[:, (cur + k0) * 8:
                                            (cur + k0 + kt) * 8],
                                num_idxs=kt * P,
                                num_idxs_reg=kt * P,
                                elem_size=elem,
                                queue_num=next_q(),
                            )
                        ohb = build_onehots(ohp, j, cur, tp)
                        aggp = [pp.tile([c1 - c0, P], F32, tag=f"agg{k}",
                                        name=f"agg{k}")
                                for k, (c0, c1) in enumerate(chunks)]
                        for t in range(tp):
                            for k, (c0, c1) in enumerate(chunks):
                                nc.tensor.matmul(
                                    out=aggp[k][:], lhsT=msg[:, t, c0:c1],
                                    rhs=ohb[:, t, :],
                                    start=(t == 0), stop=(t == tp - 1))
                        jsl = slice(j * P, (j + 1) * P)
                        if not pass_b:
                            for k in range(len(chunks)):
                                nc.scalar.copy(out=acc[k][:, jsl],
                                               in_=aggp[k][:])
                        else:
                            aggs = []
                            for k, (c0, c1) in enumerate(chunks):
                                s = asp.tile([c1 - c0, P], BF16,
                                             tag=f"aggs{k}", name=f"aggs{k}")
                                nc.vector.tensor_tensor(
                                    out=s[:], in0=aggp[k][:],
                                    in1=acc[k][:, jsl], op=add)
                                aggs.append(s[:])
                            epilogue(j, aggs)
                            if post_block is not None:
                                post_block(j)

                halfpass(False)
                halfpass(True)

        def gemm_epilogue(ep, epp, j, aggs, wtiles, bsb, od, relu_out, dst):
            """hps = aggs^T @ W (+psum); table row = dinv*relu(dinv*hps + b).

            The outer dinv folds the NEXT layer's src factor into the table
            (dinv>0 commutes with relu via the activation scale operand)."""
            hps = epp.tile([P, od], F32, tag="hp")
            for k, a in enumerate(aggs):
                nc.tensor.matmul(out=hps[:], lhsT=a, rhs=wtiles[k][:],
                                 start=(k == 0), stop=(k == len(aggs) - 1))
            pre = ep.tile([P, od], F32, tag="pre")
            nc.vector.scalar_tensor_tensor(
                out=pre[:], in0=hps[:], scalar=dvs[:, j:j + 1],
                in1=bsb[:], op0=mul, op1=add)
            hsb = ep.tile([P, od], BF16, tag="h")
            nc.scalar.activation(out=hsb[:], in_=pre[:], func=relu,
                                 scale=dvs[:, j:j + 1])
            nc.sync.dma_start(out=dst[j * P:(j + 1) * P, :], in_=hsb[:])

        def ag_a(src, dsta):
            nc.gpsimd.collective_compute(
                "AllGather", mybir.AluOpType.bypass, replica_groups=rg,
                ins=[src[0:cfg.ha, :].opt()], outs=[dsta[:, :].opt()])

        def ag_b(src, dstb):
            nc.gpsimd.collective_compute(
                "AllGather", mybir.AluOpType.bypass, replica_groups=rg,
                ins=[src[cfg.ha:cfg.slab, :].opt()], outs=[dstb[:, :].opt()])

        def make_post(src, dsta):
            def post(j):
                if j == cfg.bpc_a - 1:
                    ag_a(src, dsta)
            return post

        # ------ layer 1: host-aggregated agg1(2) -> h1(d1), GEMM only ------
        with tc.tile_pool(name="l1e", bufs=2) as ep, \
             tc.tile_pool(name="l1ep", bufs=2, space="PSUM") as epp:
            for j in range(bpc):
                gemm_epilogue(ep, epp, j, [ag1s[:, j * P:(j + 1) * P]],
                              [w1s], b1sb, d1, True, ag1)
                if j == cfg.bpc_a - 1:
                    ag_a(ag1, t1a)
        ag_b(ag1, t1b)

        if debug_layers >= 2:
            # ---------------- layer 2: h1(d1) -> h2(d2) -----------------------
            with tc.tile_pool(name="l2e", bufs=2) as ep, \
                 tc.tile_pool(name="l2ep", bufs=2, space="PSUM") as epp:
                def epi2(j, aggs):
                    gemm_epilogue(ep, epp, j, aggs, [w2s], b2sb, d2, True, ag2)
                seg_blocks("l2", t1a[:, :], t1b[:, :], d1, d1, epi2,
                           post_block=make_post(ag2, t2a))
            ag_b(ag2, t2b)

        if debug_layers >= 3:
            # ------- layer 3: h2(d2) -> h3(d3) -> t = h3@W4 (d4) --------------
            with tc.tile_pool(name="l3e", bufs=2) as ep, \
                 tc.tile_pool(name="l3ep", bufs=1, space="PSUM") as epp:
                def epi3(j, aggs):
                    hps = epp.tile([P, d3], F32, tag="hp")
                    for k in range(d2 // P):
                        nc.tensor.matmul(out=hps[:], lhsT=aggs[k], rhs=w3s[k][:],
                                         start=(k == 0), stop=(k == d2 // P - 1))
                    pre = ep.tile([P, d3], F32, tag="pre")
                    nc.vector.scalar_tensor_tensor(
                        out=pre[:], in0=hps[:], scalar=dvs[:, j:j + 1],
                        in1=b3sb[:], op0=mul, op1=add)
                    hsb = ep.tile([P, d3], BF16, tag="h")
                    nc.scalar.activation(out=hsb[:], in_=pre[:], func=relu)
                    htp = epp.tile([P, d3], BF16, tag="htp")
                    for k in range(d3 // P):
                        nc.tensor.transpose(out=htp[:, k * P:(k + 1) * P],
                                            in_=hsb[:, k * P:(k + 1) * P],
                                            identity=ident[:])
                    hts = ep.tile([P, d3], BF16, tag="hts")
                    nc.scalar.copy(out=hts[:], in_=htp[:])
                    tps = epp.tile([P, d4], F32, tag="tp")
                    for k in range(d3 // P):
                        nc.tensor.matmul(out=tps[:], lhsT=hts[:, k * P:(k + 1) * P],
                                         rhs=w4s[k][:],
                                         start=(k == 0), stop=(k == d3 // P - 1))
                    # t-table row = dinv_v * t_v (src factor for L4 gathers)
                    tsb = ep.tile([P, cfg.tpad], BF16, tag="t")
                    nc.vector.tensor_scalar(out=tsb[:, 0:d4], in0=tps[:],
                                            scalar1=dvs[:, j:j + 1],
                                            scalar2=None, op0=mul)
                    nc.sync.dma_start(out=agt[j * P:(j + 1) * P, :], in_=tsb[:])
                seg_blocks("l3", t2a[:, :], t2b[:, :], d2, d2, epi3,
                           post_block=make_post(agt, tta))
            ag_b(agt, ttb)

        if debug_layers >= 4:
            # ---------------- layer 4: t(d4) -> out ---------------------------
            with tc.tile_pool(name="l4e", bufs=2) as ep, \
                 tc.tile_pool(name="l4ep", bufs=2, space="PSUM") as epp:
                def epi4(j, aggs):
                    ops = epp.tile([P, d4], F32, tag="op")
                    nc.tensor.matmul(out=ops[:], lhsT=aggs[0], rhs=id2[:],
                                     start=True, stop=True)
                    # out = dinv_v * agg + b4  (no relu on the last layer)
                    nc.vector.scalar_tensor_tensor(
                        out=out_acc[:, j * d4:(j + 1) * d4], in0=ops[:],
                        scalar=dvs[:, j:j + 1], in1=b4sb[:], op0=mul, op1=add)
                seg_blocks("l4", tta[:, :], ttb[:, :], cfg.tpad, d4, epi4)

        nc.sync.dma_start(
            out=out_p[:, :].rearrange("(j p) c -> p j c", p=P),
            in_=out_acc[:].rearrange("p (j c) -> p j c", c=d4))

    return nc


# --------------------------------------------------------------------------
# Entry points
# --------------------------------------------------------------------------

def make_in_maps(cfg, per_core, W1, b1, W2, b2, W3, b3, W4, b4):
    shared = {
        "W1": np.ascontiguousarray(W1, np.float32),
        "W2": np.ascontiguousarray(W2, np.float32),
        "W3": np.ascontiguousarray(W3, np.float32),
        "W4": np.ascontiguousarray(W4, np.float32),
        "b1": np.ascontiguousarray(b1, np.float32).reshape(1, -1),
        "b2": np.ascontiguousarray(b2, np.float32).reshape(1, -1),
        "b3": np.ascontiguousarray(b3, np.float32).reshape(1, -1),
        "b4": np.ascontiguousarray(b4, np.float32).reshape(1, -1),
    }
    return [dict(per_core[c], **shared) for c in range(NCORES)]


_CACHE = {}


def _prep_and_build(cfg, x, edge_index):
    tiles, per_core = preprocess(cfg, edge_index, x)
    key = tuple(tiles.flatten().tolist())
    if key not in _CACHE:
        nc = build_module(cfg, tiles)
        nc.compile()  # Bacc pipeline (reg alloc etc.) before serialization
        _CACHE[key] = nc
    return _CACHE[key], per_core


def _enable_tracing():
    """Make trace=True work in this container: synthesize antenv.axon_hooks
    (the boot image lacks it), register the ctypes NTFF hook, and neuter the
    cloud artifact upload."""
    import types
    import concourse.bass_utils as bu
    try:
        import antenv.axon_hooks  # noqa: F401
    except ImportError:
        import antenv
        mod = types.ModuleType("antenv.axon_hooks")
        holder = {"h": None}
        mod.set_axon_ntff_profile_hook = lambda h: holder.__setitem__("h", h)
        mod.get_axon_ntff_profile_hook = lambda: holder["h"]
        sys.modules["antenv.axon_hooks"] = mod
        antenv.axon_hooks = mod
        if "/root/.axon_site" not in sys.path:
            sys.path.insert(0, "/root/.axon_site")
        from trn_agent_boot.trn_boot import _ntff_profile_via_ctypes
        h = _ntff_profile_via_ctypes("/opt/axon/libaxon_pjrt.so")
        if h is not None:
            mod.set_axon_ntff_profile_hook(h)
    bu.upload_artifacts = lambda tmpdir: tmpdir


def run_on_hw(inputs, trace=False):
    from concourse.bass_utils import run_bass_kernel_spmd
    if trace:
        _enable_tracing()
    cfg = REAL_CFG
    x = np.asarray(inputs["x"], np.float32)
    nc, per_core = _prep_and_build(cfg, x, np.asarray(inputs["edge_index"]))
    in_maps = make_in_maps(cfg, per_core,
                           inputs["W1"], inputs["b1"], inputs["W2"],
                           inputs["b2"], inputs["W3"], inputs["b3"],
                           inputs["W4"], inputs["b4"])
    res = run_bass_kernel_spmd(nc, in_maps, core_ids=list(range(NCORES)),
                               trace=trace)
    out = np.concatenate([res.results[c]["out"] for c in range(NCORES)],
                         axis=0)[:cfg.n_nodes]
    return out.astype(np.float32), res


def kernel(x, edge_index, W1, b1, W2, b2, W3, b3, W4, b4):
    out, _ = run_on_hw(dict(x=x, edge_index=edge_index, W1=W1, b1=b1, W2=W2,
                            b2=b2, W3=W3, b3=b3, W4=W4, b4=b4))
    return out


# revision 20
# speedup vs baseline: 1.0660x; 1.0660x over previous
"""Trainium2 Bass kernel for a 4-layer GCN (PyG GCNConv semantics).

Math: each layer computes  h' = relu(A_hat @ h @ W + b)  where
A_hat = D^-1/2 A D^-1/2 + D^-1 (self loops), D = in-degree + 1.
Aggregation commutes with the dense transform, so each layer aggregates in
whichever of (in_dim, out_dim) is cheaper:
  L1: aggregate x (width 2, host-permuted stream), then @W1      -> h1 [N,128]
  L2: gather h1 rows (256B bf16), segment-sum, @W2               -> h2 [N,256]
  L3: gather h2 rows (512B bf16), segment-sum, @W3, fuse t=h3@W4 -> t  [N,2]
  L4: gather t rows (256B bf16 padded), segment-sum, + b4        -> out [N,2]

Normalization is separable: w_uv = dinv[u]*dinv[v].  Tables store
dinv[u]*h[u] (src factor folded in at the producing epilogue); the dst
factor dinv[v] is applied post-GEMM per block (diag-left commutes with @W).
Self-loops are then plain edges.  One-hot scatter matrices are pure binary
(iota == dstslot), built batched on DVE; pad slots use dstslot=-1.

Sharding: destination-node slabs. Core c owns 49 blocks x 128 dst nodes.
Edges (+self loops) are grouped per dst block, split lo/hi at src<32768
(dma_gather idx is int16), sorted by src, padded to 128-edge tiles.

Gathers run on all 4 SWDGE queues round-robin (4 Q7 pairs generate
descriptors concurrently; ~2.9x descgen throughput vs one queue).
Tables are bf16 (halves gather bytes); PSUM accumulation stays fp32.
Tile counts per (block, phase) are max'd across the 8 cores so one NEFF
serves all cores (SPMD).
"""

import sys

for _p in ("/opt/trn_rl_repo",):
    if _p not in sys.path:
        sys.path.insert(0, _p)

from contextlib import ExitStack

import numpy as np

import concourse.bass as bass
import concourse.bacc as bacc
import concourse.mybir as mybir
import concourse.tile as tile
from concourse import library_config
from concourse.masks import make_identity

P = 128
NCORES = 8
F32 = mybir.dt.float32
BF16 = mybir.dt.bfloat16
I16 = mybir.dt.int16
I32 = mybir.dt.int32
NQ = 4  # SWDGE queues


class GCNConfig:
    def __init__(self, n_nodes, dims, blocks_per_core, bpc_a):
        self.n_nodes = n_nodes
        self.dims = list(dims)  # [2, 128, 256, 512, 2]
        self.bpc = blocks_per_core
        self.bpc_a = bpc_a          # blocks in slab half A (rest in half B)
        self.slab = blocks_per_core * P
        self.npad = NCORES * self.slab
        self.ha = bpc_a * P         # rows per core in half A
        self.hb = self.slab - self.ha
        self.na = NCORES * self.ha  # rows in table A (= concat of A halves)
        self.nb = NCORES * self.hb
        assert self.npad >= n_nodes
        assert self.na < 32768 and self.nb < 32768  # int16 gather indices
        self.tpad = 128  # bf16 cols in the padded width-2 "t" table (256B row)


REAL_CFG = GCNConfig(n_nodes=50000, dims=[2, 128, 256, 512, 2],
                     blocks_per_core=49, bpc_a=24)


# --------------------------------------------------------------------------
# Host-side graph preprocessing
# --------------------------------------------------------------------------

def preprocess(cfg, edge_index, x):
    """Shard + tile the graph. Returns (tiles [bpc,2] int, per-core dict of
    host arrays)."""
    src = np.asarray(edge_index[0], dtype=np.int64)
    dst = np.asarray(edge_index[1], dtype=np.int64)
    n = cfg.n_nodes
    deg = np.bincount(dst, minlength=n).astype(np.float32) + 1.0
    dinv = 1.0 / np.sqrt(deg)
    dinv_pad = np.ones(cfg.npad, np.float32)
    dinv_pad[:n] = dinv

    es = np.concatenate([src, np.arange(n, dtype=np.int64)])
    ed = np.concatenate([dst, np.arange(n, dtype=np.int64)])

    blk = ed // P
    owner = es // cfg.slab
    r_in_slab = es % cfg.slab
    hi = (r_in_slab >= cfg.ha).astype(np.int64)
    # row index within half-table A or B (tables are concat of slab halves)
    erow = np.where(hi == 1, owner * cfg.hb + (r_in_slab - cfg.ha),
                    owner * cfg.ha + r_in_slab)
    order = np.lexsort((es, hi, blk))
    es, ed, blk, hi, erow = (es[order], ed[order], blk[order], hi[order],
                             erow[order])

    nblocks = NCORES * cfg.bpc
    cnt = np.zeros((nblocks, 2), np.int64)
    np.add.at(cnt, (blk, hi), 1)
    cnt_core = cnt.reshape(NCORES, cfg.bpc, 2)
    tiles = (-(-cnt_core // P)).max(axis=0)  # [bpc, 2] ceil-div then max
    tiles = np.maximum(tiles, 1)  # both phases always present
    tt = int(tiles.sum())

    # group start offsets in the sorted edge arrays, per (block, phase)
    starts = np.zeros(nblocks * 2 + 1, np.int64)
    np.add.at(starts, blk * 2 + hi + 1, 1)
    starts = np.cumsum(starts)

    # L1 aggregation on host (width-2 segment-sum; same class of host prep
    # as the per-edge x[src] gather the stream variant needed):
    #   agg1[v] = sum_{e in E+self, dst=v} dinv[src] * x[src]
    xf = np.asarray(x, dtype=np.float32)
    xw = xf * dinv[:, None]  # [n, 2]
    agg1 = np.zeros((cfg.npad, 2), np.float32)
    np.add.at(agg1, ed, xw[es])

    per_core = []
    for c in range(NCORES):
        idx = np.zeros((tt, P), np.int16)
        dl = np.full((tt, P), -1.0, np.float32)  # -1 => zero one-hot col
        cur = 0
        for j in range(cfg.bpc):
            g = c * cfg.bpc + j
            for ph in range(2):
                t = int(tiles[j, ph])
                if t == 0:
                    continue
                s0, s1 = starts[g * 2 + ph], starts[g * 2 + ph + 1]
                ne = s1 - s0
                assert ne <= t * P
                flat_i = np.zeros(t * P, np.int64)
                flat_d = np.full(t * P, -1.0, np.float32)
                flat_i[:ne] = erow[s0:s1]
                flat_d[:ne] = (ed[s0:s1] - g * P).astype(np.float32)
                sl = slice(cur, cur + t)
                idx[sl] = flat_i.reshape(t, P).astype(np.int16)
                dl[sl] = flat_d.reshape(t, P)
                cur += t
        assert cur == tt
        # device layouts:
        # idx: [128, tt*8] int16 -- per tile, idx i at [i%16 (replic 8x), i//16]
        idx_dev = np.zeros((P, tt * 8), np.int16)
        for rep in range(8):
            idx_dev[rep * 16:(rep + 1) * 16] = (
                idx.reshape(tt, 8, 16).transpose(2, 0, 1).reshape(16, tt * 8))
        # dinv columns for this core's slab: [128, bpc]
        dslab = dinv_pad[c * cfg.slab:(c + 1) * cfg.slab].reshape(cfg.bpc, P).T
        per_core.append({
            "eidx": idx_dev,
            "edl": dl.T.copy(),                      # [128, tt]
            "eag1": np.ascontiguousarray(
                agg1[c * cfg.slab:(c + 1) * cfg.slab].T),  # [2, slab]
            "dinv": np.ascontiguousarray(dslab),     # [128, bpc]
        })
    return tiles, per_core


# --------------------------------------------------------------------------
# Device module builder
# --------------------------------------------------------------------------

def build_module(cfg, tiles, debug_layers=4):
    d0, d1, d2, d3, d4 = cfg.dims
    bpc = cfg.bpc
    tt = int(tiles.sum())
    nc = bacc.Bacc(None, target_bir_lowering=False, num_swdge_queues=NQ)

    eidx = nc.declare_dram_parameter("eidx", [P, tt * 8], I16, False)
    edl = nc.declare_dram_parameter("edl", [P, tt], F32, False)
    eag1 = nc.declare_dram_parameter("eag1", [d0, cfg.slab], F32, False)
    dinv_p = nc.declare_dram_parameter("dinv", [P, bpc], F32, False)
    W1 = nc.declare_dram_parameter("W1", [d0, d1], F32, False)
    W2 = nc.declare_dram_parameter("W2", [d1, d2], F32, False)
    W3 = nc.declare_dram_parameter("W3", [d2, d3], F32, False)
    W4 = nc.declare_dram_parameter("W4", [d3, d4], F32, False)
    b1 = nc.declare_dram_parameter("b1", [1, d1], F32, False)
    b2 = nc.declare_dram_parameter("b2", [1, d2], F32, False)
    b3 = nc.declare_dram_parameter("b3", [1, d3], F32, False)
    b4 = nc.declare_dram_parameter("b4", [1, d4], F32, False)
    out_p = nc.declare_dram_parameter("out", [cfg.slab, d4], F32, True)

    rg = [list(range(NCORES))]
    eq = mybir.AluOpType.is_equal
    mul = mybir.AluOpType.mult
    add = mybir.AluOpType.add
    relu = mybir.ActivationFunctionType.Relu

    qctr = [0]

    def next_q():
        q = qctr[0] % NQ
        qctr[0] += 1
        return q

    with tile.TileContext(nc, num_cores=NCORES) as tc, ExitStack() as ctx:
        dram = ctx.enter_context(tc.tile_pool(name="dram", bufs=1, space="DRAM"))
        t1a = dram.tile([cfg.na, d1], BF16, addr_space="Shared")
        t1b = dram.tile([cfg.nb, d1], BF16, addr_space="Shared")
        t2a = dram.tile([cfg.na, d2], BF16, addr_space="Shared")
        t2b = dram.tile([cfg.nb, d2], BF16, addr_space="Shared")
        tta = dram.tile([cfg.na, cfg.tpad], BF16, addr_space="Shared")
        ttb = dram.tile([cfg.nb, cfg.tpad], BF16, addr_space="Shared")
        ag1 = dram.tile([cfg.slab, d1], BF16)
        ag2 = dram.tile([cfg.slab, d2], BF16)
        agt = dram.tile([cfg.slab, cfg.tpad], BF16)

        const = ctx.enter_context(tc.tile_pool(name="const", bufs=1))
        iota_i = const.tile([P, P], I32)
        iota_f = const.tile([P, P], F32)
        nc.gpsimd.iota(iota_i[:], pattern=[[1, P]], base=0, channel_multiplier=0)
        nc.vector.tensor_copy(iota_f[:], iota_i[:])
        iota_rep = const.tile([P, 8 * P], F32)
        for k in range(8):
            nc.vector.tensor_copy(iota_rep[:, k * P:(k + 1) * P], iota_f[:])
        ident = const.tile([P, P], BF16)
        make_identity(nc, ident[:])
        id2 = const.tile([d4, d4], BF16)
        make_identity(nc, id2[:])
        ones = const.tile([1, P], BF16)
        nc.vector.memset(ones[:], 1.0)
        # iota (standard lib) is done; switch GPSIMD ucode to the library
        # that provides DMAGatherAnt
        nc.gpsimd.load_library(library_config.mlp)

        # weights as bf16 in SBUF
        w1s = const.tile([d0, d1], BF16)
        nc.gpsimd.dma_start(out=w1s[:], in_=W1[:, :])
        w2s = const.tile([d1, d2], BF16)
        nc.gpsimd.dma_start(out=w2s[:], in_=W2[:, :])
        w3s = [const.tile([P, d3], BF16, tag=f"w3_{k}", name=f"w3_{k}") for k in range(d2 // P)]
        for k in range(d2 // P):
            nc.gpsimd.dma_start(out=w3s[k][:], in_=W3[k * P:(k + 1) * P, :])
        w4s = [const.tile([P, d4], BF16, tag=f"w4_{k}", name=f"w4_{k}") for k in range(d3 // P)]
        for k in range(d3 // P):
            nc.gpsimd.dma_start(out=w4s[k][:], in_=W4[k * P:(k + 1) * P, :])
        # bias rows (bf16 for the ones-matmul trick)
        brs = []
        for name, bparam, od in (("b1", b1, d1), ("b2", b2, d2),
                                 ("b3", b3, d3), ("b4", b4, d4)):
            r = const.tile([1, od], BF16, tag=name + "s", name=name + "s")
            nc.gpsimd.dma_start(out=r[:], in_=bparam[:, :])
            brs.append(r)
        b1r, b2r, b3r, b4r = brs
        # bias broadcast to all partitions (fp32 SBUF), one per layer
        bsbs = []
        with tc.tile_pool(name="bp", bufs=1, space="PSUM") as bpp:
            for name, br, od in (("b1", b1r, d1), ("b2", b2r, d2),
                                 ("b3", b3r, d3), ("b4", b4r, d4)):
                ps = bpp.tile([P, od], F32, tag="bps", name=f"bps_{name}")
                nc.tensor.matmul(out=ps[:], lhsT=ones[:], rhs=br[:],
                                 start=True, stop=True)
                sb = const.tile([P, od], F32, tag=name + "sb", name=name + "sb")
                nc.scalar.copy(out=sb[:], in_=ps[:])
                bsbs.append(sb)
        b1sb, b2sb, b3sb, b4sb = bsbs

        dvs = const.tile([P, bpc], F32)
        nc.sync.dma_start(out=dvs[:], in_=dinv_p[:, :])
        esb = const.tile([P, tt * 8], I16)
        nc.sync.dma_start(out=esb[:], in_=eidx[:, :])
        dls = const.tile([P, tt], F32)
        nc.sync.dma_start(out=dls[:], in_=edl[:, :])
        ag1s = const.tile([d0, cfg.slab], BF16)
        nc.gpsimd.dma_start(out=ag1s[:], in_=eag1[:, :])

        out_acc = const.tile([P, d4 * bpc], F32)
        nc.vector.memset(out_acc[:], 0.0)

        def build_onehots(ohp, j, cur, t_all):
            """Batched binary one-hot tiles for block j: [128, t_all*128] bf16."""
            ohb = ohp.tile([P, t_all, P], BF16, tag="ohb")
            for g0 in range(0, t_all, 8):
                g = min(8, t_all - g0)
                nc.vector.tensor_tensor(
                    out=ohb[:, g0:g0 + g, :],
                    in0=iota_rep[:].rearrange("p (n q) -> p n q", q=P)[:, 0:g, :],
                    in1=dls[:, cur + g0:cur + g0 + g]
                        .rearrange("p (n u) -> p n u", u=1).to_broadcast([P, g, P]),
                    op=eq)
            return ohb

        # per-block tile offsets into the shared (eidx, dls) streams
        boffs = []
        _cur = 0
        for j in range(bpc):
            _tlo, _thi = int(tiles[j, 0]), int(tiles[j, 1])
            boffs.append((_cur, _tlo, _thi))
            _cur += _tlo + _thi

        def seg_blocks(lname, taba, tabb, elem, icols, epilogue,
                       post_block=None, msg_bufs=6):
            """Two passes over dst blocks: pass A gathers from the half-A
            table, accumulating per block into an SBUF buffer; pass B gathers
            from half-B, adds, and runs the epilogue.  Pass A only depends on
            AG_A of the previous layer, so AG_B overlaps pass A."""
            chunks = [(k * P, min(icols, (k + 1) * P))
                      for k in range(-(-icols // P))]
            with tc.tile_pool(name=lname + "m", bufs=msg_bufs) as mp, \
                 tc.tile_pool(name=lname + "oh", bufs=3) as ohp, \
                 tc.tile_pool(name=lname + "ps", bufs=2, space="PSUM") as pp, \
                 tc.tile_pool(name=lname + "ac", bufs=1) as accp, \
                 tc.tile_pool(name=lname + "as", bufs=2) as asp:
                acc = [accp.tile([c1 - c0, bpc * P], F32, tag=f"acc{k}",
                                 name=f"acc{k}")
                       for k, (c0, c1) in enumerate(chunks)]

                def halfpass(pass_b):
                    for j in range(bpc):
                        cur0, tlo, thi = boffs[j]
                        cur = cur0 + (tlo if pass_b else 0)
                        tp = thi if pass_b else tlo
                        tab = tabb if pass_b else taba
                        msg = mp.tile([P, tp, elem], BF16,
                                      tag="msgB" if pass_b else "msgA")
                        # HW: >512 idxs per DMAGatherAnt wedges the device
                        # (SWDGE ring limit); chunk to 4 tiles.
                        for k0 in range(0, tp, 4):
                            kt = min(4, tp - k0)
                            nc.gpsimd.dma_gather(
                                out_ap=msg[:, k0:k0 + kt, :],
                                in_ap=tab,
                                idxs_ap=esb[:, (cur + k0) * 8:
                                            (cur + k0 + kt) * 8],
                                num_idxs=kt * P,
                                num_idxs_reg=kt * P,
                                elem_size=elem,
                                queue_num=next_q(),
                            )
                        ohb = build_onehots(ohp, j, cur, tp)
                        aggp = [pp.tile([c1 - c0, P], F32, tag=f"agg{k}",
                                        name=f"agg{k}")
                                for k, (c0, c1) in enumerate(chunks)]
                        for t in range(tp):
                            for k, (c0, c1) in enumerate(chunks):
                                nc.tensor.matmul(
                                    out=aggp[k][:], lhsT=msg[:, t, c0:c1],
                                    rhs=ohb[:, t, :],
                                    start=(t == 0), stop=(t == tp - 1))
                        jsl = slice(j * P, (j + 1) * P)
                        if not pass_b:
                            for k in range(len(chunks)):
                                nc.scalar.copy(out=acc[k][:, jsl],
                                               in_=aggp[k][:])
                        else:
                            aggs = []
                            for k, (c0, c1) in enumerate(chunks):
                                s = asp.tile([c1 - c0, P], BF16,
                                             tag=f"aggs{k}", name=f"aggs{k}")
                                nc.vector.tensor_tensor(
                                    out=s[:], in0=aggp[k][:],
                                    in1=acc[k][:, jsl], op=add)
                                aggs.append(s[:])
                            epilogue(j, aggs)
                            if post_block is not None:
                                post_block(j)

                halfpass(False)
                halfpass(True)

        def gemm_epilogue(ep, epp, j, aggs, wtiles, bsb, od, relu_out, dst):
            """hps = aggs^T @ W (+psum); table row = dinv*relu(dinv*hps + b).

            The outer dinv folds the NEXT layer's src factor into the table
            (dinv>0 commutes with relu via the activation scale operand)."""
            hps = epp.tile([P, od], F32, tag="hp")
            for k, a in enumerate(aggs):
                nc.tensor.matmul(out=hps[:], lhsT=a, rhs=wtiles[k][:],
                                 start=(k == 0), stop=(k == len(aggs) - 1))
            pre = ep.tile([P, od], F32, tag="pre")
            nc.vector.scalar_tensor_tensor(
                out=pre[:], in0=hps[:], scalar=dvs[:, j:j + 1],
                in1=bsb[:], op0=mul, op1=add)
            hsb = ep.tile([P, od], BF16, tag="h")
            nc.scalar.activation(out=hsb[:], in_=pre[:], func=relu,
                                 scale=dvs[:, j:j + 1])
            nc.sync.dma_start(out=dst[j * P:(j + 1) * P, :], in_=hsb[:])

        def ag_a(src, dsta):
            nc.gpsimd.collective_compute(
                "AllGather", mybir.AluOpType.bypass, replica_groups=rg,
                ins=[src[0:cfg.ha, :].opt()], outs=[dsta[:, :].opt()])

        def ag_b(src, dstb):
            nc.gpsimd.collective_compute(
                "AllGather", mybir.AluOpType.bypass, replica_groups=rg,
                ins=[src[cfg.ha:cfg.slab, :].opt()], outs=[dstb[:, :].opt()])

        def make_post(src, dsta):
            def post(j):
                if j == cfg.bpc_a - 1:
                    ag_a(src, dsta)
            return post

        # ------ layer 1: host-aggregated agg1(2) -> h1(d1), GEMM only ------
        with tc.tile_pool(name="l1e", bufs=2) as ep, \
             tc.tile_pool(name="l1ep", bufs=2, space="PSUM") as epp:
            for j in range(bpc):
                gemm_epilogue(ep, epp, j, [ag1s[:, j * P:(j + 1) * P]],
                              [w1s], b1sb, d1, True, ag1)
                if j == cfg.bpc_a - 1:
                    ag_a(ag1, t1a)
        ag_b(ag1, t1b)

        if debug_layers >= 2:
            # ---------------- layer 2: h1(d1) -> h2(d2) -----------------------
            with tc.tile_pool(name="l2e", bufs=2) as ep, \
                 tc.tile_pool(name="l2ep", bufs=2, space="PSUM") as epp:
                def epi2(j, aggs):
                    gemm_epilogue(ep, epp, j, aggs, [w2s], b2sb, d2, True, ag2)
                seg_blocks("l2", t1a[:, :], t1b[:, :], d1, d1, epi2,
                           post_block=make_post(ag2, t2a))
            ag_b(ag2, t2b)

        if debug_layers >= 3:
            # ------- layer 3: h2(d2) -> h3(d3) -> t = h3@W4 (d4) --------------
            with tc.tile_pool(name="l3e", bufs=2) as ep, \
                 tc.tile_pool(name="l3ep", bufs=1, space="PSUM") as epp:
                def epi3(j, aggs):
                    hps = epp.tile([P, d3], F32, tag="hp")
                    for k in range(d2 // P):
                        nc.tensor.matmul(out=hps[:], lhsT=aggs[k], rhs=w3s[k][:],
                                         start=(k == 0), stop=(k == d2 // P - 1))
                    pre = ep.tile([P, d3], F32, tag="pre")
                    nc.vector.scalar_tensor_tensor(
                        out=pre[:], in0=hps[:], scalar=dvs[:, j:j + 1],
                        in1=b3sb[:], op0=mul, op1=add)
                    hsb = ep.tile([P, d3], BF16, tag="h")
                    nc.scalar.activation(out=hsb[:], in_=pre[:], func=relu)
                    htp = epp.tile([P, d3], BF16, tag="htp")
                    for k in range(d3 // P):
                        nc.tensor.transpose(out=htp[:, k * P:(k + 1) * P],
                                            in_=hsb[:, k * P:(k + 1) * P],
                                            identity=ident[:])
                    hts = ep.tile([P, d3], BF16, tag="hts")
                    nc.scalar.copy(out=hts[:], in_=htp[:])
                    tps = epp.tile([P, d4], F32, tag="tp")
                    for k in range(d3 // P):
                        nc.tensor.matmul(out=tps[:], lhsT=hts[:, k * P:(k + 1) * P],
                                         rhs=w4s[k][:],
                                         start=(k == 0), stop=(k == d3 // P - 1))
                    # t-table row = dinv_v * t_v (src factor for L4 gathers)
                    tsb = ep.tile([P, cfg.tpad], BF16, tag="t")
                    nc.vector.tensor_scalar(out=tsb[:, 0:d4], in0=tps[:],
                                            scalar1=dvs[:, j:j + 1],
                                            scalar2=None, op0=mul)
                    nc.sync.dma_start(out=agt[j * P:(j + 1) * P, :], in_=tsb[:])
                seg_blocks("l3", t2a[:, :], t2b[:, :], d2, d2, epi3,
                           post_block=make_post(agt, tta))
            ag_b(agt, ttb)

        if debug_layers >= 4:
            # ---------------- layer 4: t(d4) -> out ---------------------------
            with tc.tile_pool(name="l4e", bufs=2) as ep, \
                 tc.tile_pool(name="l4ep", bufs=2, space="PSUM") as epp:
                def epi4(j, aggs):
                    ops = epp.tile([P, d4], F32, tag="op")
                    nc.tensor.matmul(out=ops[:], lhsT=aggs[0], rhs=id2[:],
                                     start=True, stop=True)
                    # out = dinv_v * agg + b4  (no relu on the last layer)
                    nc.vector.scalar_tensor_tensor(
                        out=out_acc[:, j * d4:(j + 1) * d4], in0=ops[:],
                        scalar=dvs[:, j:j + 1], in1=b4sb[:], op0=mul, op1=add)
                seg_blocks("l4", tta[:, :], ttb[:, :], cfg.tpad, d4, epi4)

        nc.sync.dma_start(
            out=out_p[:, :].rearrange("(j p) c -> p j c", p=P),
            in_=out_acc[:].rearrange("p (j c) -> p j c", c=d4))

    return nc


# --------------------------------------------------------------------------
# Entry points
# --------------------------------------------------------------------------

def make_in_maps(cfg, per_core, W1, b1, W2, b2, W3, b3, W4, b4):
    shared = {
        "W1": np.ascontiguousarray(W1, np.float32),
        "W2": np.ascontiguousarray(W2, np.float32),
        "W3": np.ascontiguousarray(W3, np.float32),
        "W4": np.ascontiguousarray(W4, np.float32),
        "b1": np.ascontiguousarray(b1, np.float32).reshape(1, -1),
        "b2": np.ascontiguousarray(b2, np.float32).reshape(1, -1),
        "b3": np.ascontiguousarray(b3, np.float32).reshape(1, -1),
        "b4": np.ascontiguousarray(b4, np.float32).reshape(1, -1),
    }
    return [dict(per_core[c], **shared) for c in range(NCORES)]


_CACHE = {}


def _prep_and_build(cfg, x, edge_index):
    tiles, per_core = preprocess(cfg, edge_index, x)
    key = tuple(tiles.flatten().tolist())
    if key not in _CACHE:
        nc = build_module(cfg, tiles)
        nc.compile()  # Bacc pipeline (reg alloc etc.) before serialization
        _CACHE[key] = nc
    return _CACHE[key], per_core


def _enable_tracing():
    """Make trace=True work in this container: synthesize antenv.axon_hooks
    (the boot image lacks it), register the ctypes NTFF hook, and neuter the
    cloud artifact upload."""
    import types
    import concourse.bass_utils as bu
    try:
        import antenv.axon_hooks  # noqa: F401
    except ImportError:
        import antenv
        mod = types.ModuleType("antenv.axon_hooks")
        holder = {"h": None}
        mod.set_axon_ntff_profile_hook = lambda h: holder.__setitem__("h", h)
        mod.get_axon_ntff_profile_hook = lambda: holder["h"]
        sys.modules["antenv.axon_hooks"] = mod
        antenv.axon_hooks = mod
        if "/root/.axon_site" not in sys.path:
            sys.path.insert(0, "/root/.axon_site")
        from trn_agent_boot.trn_boot import _ntff_profile_via_ctypes
        h = _ntff_profile_via_ctypes("/opt/axon/libaxon_pjrt.so")
        if h is not None:
            mod.set_axon_ntff_profile_hook(h)
    bu.upload_artifacts = lambda tmpdir: tmpdir


def run_on_hw(inputs, trace=False):
    from concourse.bass_utils import run_bass_kernel_spmd
    if trace:
        _enable_tracing()
    cfg = REAL_CFG
    x = np.asarray(inputs["x"], np.float32)
    nc, per_core = _prep_and_build(cfg, x, np.asarray(inputs["edge_index"]))
    in_maps = make_in_maps(cfg, per_core,
                           inputs["W1"], inputs["b1"], inputs["W2"],
                           inputs["b2"], inputs["W3"], inputs["b3"],
                           inputs["W4"], inputs["b4"])
    res = run_bass_kernel_spmd(nc, in_maps, core_ids=list(range(NCORES)),
                               trace=trace)
    out = np.concatenate([res.results[c]["out"] for c in range(NCORES)],
                         axis=0)[:cfg.n_nodes]
    return out.astype(np.float32), res


def kernel(x, edge_index, W1, b1, W2, b2, W3, b3, W4, b4):
    out, _ = run_on_hw(dict(x=x, edge_index=edge_index, W1=W1, b1=b1, W2=W2,
                            b2=b2, W3=W3, b3=b3, W4=W4, b4=b4))
    return out
